# revision 47
# baseline (speedup 1.0000x reference)
"""Trainium2 Bass kernel for nn_MultiHeadAttention (B=2, S=2048, D=2048, H=16, HD=128).

Sharding: tensor-parallel across heads; each of the 8 cores owns 2 heads.

Per core:
  - QKV projection for its heads (bf16 matmuls, fp32 PSUM accumulation,
    K-contiguous, N=512); RoPE on DVE with host-precomputed cos /
    sign-folded sin tables (bf16).
  - Causal attention in transposed layout scores^T[k, q], run as one flat
    software pipeline across all chunks and heads: the PE issues score
    matmul pairs three tile-pairs ahead of the deferred attn@V + rowsum
    matmuls, ScalarE exps tile-pairs 1024 wide straight out of PSUM, and
    the DVE folds exp pairs into super-pair sums that a single ones-vector
    matmul per two pairs accumulates into the softmax denominator (fp32
    PSUM).  The whole normalize chain (rowsum copy, fast approx
    reciprocal, GpSimd partition-broadcast, DVE multiply) runs off the PE.
    The diagonal tile-pair uses a column-restricted exp / mask / attn@V
    (its low half is causally zero); causal masks are two host-precomputed
    1024-wide pair masks.
  - One AllToAll per (batch, head) redistributes head-outputs into
    sequence-slices (bf16): head 0's collective overlaps head 1's
    attention, so only a half-size collective is exposed at each batch
    tail, where it is covered by the previous batch's output projection.
    Shard layout [dest, partition, seq] makes the post-collective gather
    DMA read 1KB-contiguous runs.
  - Output projection W_o in bf16 (16-matmul fp32 accumulation groups);
    batch 1 walks the e-chunks in reverse so the W_o tiles still resident
    from batch 0 cover its first matmuls while the rest reload.
Host gathers the full output by concatenating the 8 row-slices.
"""

import numpy as np

B = 2
H = 16
HD = 128
N_CORES = 8
HEADS_PER_CORE = H // N_CORES


# ---------------------------------------------------------------- device code
def build_nc(S=2048, D=2048, n_cores=N_CORES):
    from contextlib import ExitStack

    import concourse.mybir as mybir
    import concourse.tile as tile
    from concourse import bacc

    f32 = mybir.dt.float32
    bf16 = mybir.dt.bfloat16
    Exp = mybir.ActivationFunctionType.Exp

    KT = D // 128            # contraction tiles for projections
    S2 = B * S               # total rows of x
    NCH = S // 512           # 512-wide q-chunks per batch
    SLICE = S2 // n_cores    # output rows per core
    ECH = D // 512           # 512-wide e-chunks of the output dim
    HSC = HEADS_PER_CORE
    SL8 = S // n_cores
    hd_scale = float(HD) ** -0.5

    nc = bacc.Bacc("TRN2", target_bir_lowering=False, debug=False,
                   num_devices=n_cores)

    xt = nc.dram_tensor("xt", [D, S2], bf16, kind="ExternalInput").ap()
    wqk = nc.dram_tensor("wqk", [D, 2 * HSC * 128], bf16, kind="ExternalInput").ap()
    wv = nc.dram_tensor("wv", [D, HSC * 128], bf16, kind="ExternalInput").ap()
    wo = nc.dram_tensor("wo", [H * HD, D], bf16, kind="ExternalInput").ap()
    cost = nc.dram_tensor("cost", [128, S], bf16, kind="ExternalInput").ap()
    sins = nc.dram_tensor("sins", [128, S], bf16, kind="ExternalInput").ap()
    pmA = nc.dram_tensor("pmA", [128, 2 * 512], bf16, kind="ExternalInput").ap()
    pmB = nc.dram_tensor("pmB", [128, 2 * 512], bf16, kind="ExternalInput").ap()
    ones = nc.dram_tensor("ones", [128, 1], bf16, kind="ExternalInput").ap()
    out = nc.dram_tensor("out", [SLICE, D], f32, kind="ExternalOutput").ap()

    xt_rs = xt.rearrange("(kt p) s -> p kt s", p=128)
    wqk_rs = wqk.rearrange("(kt p) m -> p kt m", p=128)
    wv_rs = wv.rearrange("(kt p) m -> p kt m", p=128)
    wo_rs = wo.rearrange("(ht p) e -> p ht e", p=128)

    with tile.TileContext(nc) as tc, ExitStack() as ctx:
        const = ctx.enter_context(tc.tile_pool(name="const", bufs=1))
        stream = ctx.enter_context(tc.tile_pool(name="stream", bufs=2))
        wop = ctx.enter_context(tc.tile_pool(name="wop", bufs=3))
        qkvp = ctx.enter_context(tc.tile_pool(name="qkvp", bufs=1))
        expp = ctx.enter_context(tc.tile_pool(name="expp", bufs=6))
        ptp = ctx.enter_context(tc.tile_pool(name="ptp", bufs=3))
        sptp = ctx.enter_context(tc.tile_pool(name="sptp", bufs=3))
        tmp = ctx.enter_context(tc.tile_pool(name="tmp", bufs=2))
        # PSUM budget (8 banks): psA 2 slots x 2 banks (qk-proj ps / score
        # pairs), psB 3 slots x 1 bank (v-proj ps / attn@V acc / o-proj ps),
        # psR 1 bank (rowsum).
        psA = ctx.enter_context(tc.tile_pool(name="psA", bufs=2, space="PSUM"))
        psB = ctx.enter_context(tc.tile_pool(name="psB", bufs=3, space="PSUM"))
        psR = ctx.enter_context(tc.tile_pool(name="psR", bufs=1, space="PSUM"))
        dram = ctx.enter_context(tc.tile_pool(name="dram", bufs=1, space="DRAM"))

        # resident constants; wqk + the first x chunk first so the PE can
        # start as early as possible
        # split the first DMAs so they spread across queues and the first
        # matmul group can start as early as possible
        wqk_sb = const.tile([128, KT, 2 * HSC * 128], bf16)
        for ct in range(2 * HSC):
            nc.sync.dma_start(wqk_sb[:, :, ct * 128:(ct + 1) * 128],
                              wqk_rs[:, :, ct * 128:(ct + 1) * 128])
        xt0_sb = stream.tile([128, KT, 512], bf16, tag="stream", name="xt0")
        nc.sync.dma_start(xt0_sb[:, 0:KT // 2, :], xt_rs[:, 0:KT // 2, 0:512])
        nc.sync.dma_start(xt0_sb[:, KT // 2:, :], xt_rs[:, KT // 2:, 0:512])
        cos_sb = const.tile([128, S], bf16)
        nc.sync.dma_start(cos_sb[:], cost[:])
        sins_sb = const.tile([128, S], bf16)
        nc.sync.dma_start(sins_sb[:], sins[:])
        wv_sb = const.tile([128, KT, HSC * 128], bf16)
        nc.sync.dma_start(wv_sb[:], wv_rs[:])
        pmA_sb = const.tile([128, 2, 512], bf16)
        nc.sync.dma_start(pmA_sb[:], pmA.rearrange("p (t q) -> p t q", t=2))
        pmB_sb = const.tile([128, 2, 512], bf16)
        nc.sync.dma_start(pmB_sb[:], pmB.rearrange("p (t q) -> p t q", t=2))
        ones_sb = const.tile([128, 1], bf16)
        nc.sync.dma_start(ones_sb[:], ones[:])

        # normalized attention output, transposed: [d, head, b*S+s] (bf16)
        attnT_sb = const.tile([128, HSC, S2], bf16)

        # the diagonal tile-pair uses a column-restricted exp, so its exp
        # buffers are partially stale; zero them once so stale bits can
        # never be NaN/Inf patterns (the pair mask multiplies them to 0)
        for i in range(6):
            exz = expp.tile([128, 2, 512], bf16, tag="ex", name=f"exz{i}")
            nc.any.memset(exz[:], 0.0)

        a2a_outs = []
        for b in range(B):
            # -------- QKV projection for batch b (heads of this core) -----
            q_sb = [qkvp.tile([128, S], bf16, tag=f"q{h}", name=f"q{h}_{b}")
                    for h in range(HSC)]
            k_sb = [qkvp.tile([128, S], bf16, tag=f"k{h}", name=f"k{h}_{b}")
                    for h in range(HSC)]
            v_sb = qkvp.tile([128, S // 128, HSC * 128], bf16, tag="v")

            for e4 in range(S // 512):
                s0 = e4 * 512
                if b == 0 and e4 == 0:
                    xt_sb = xt0_sb
                else:
                    xt_sb = stream.tile([128, KT, 512], bf16, tag="stream")
                    nc.sync.dma_start(
                        xt_sb[:], xt_rs[:, :, b * S + s0:b * S + s0 + 512])

                # q/k tiles: out^T layout [c, s], N=512
                for ct in range(2 * HSC):
                    ps = psA.tile([128, 2, 512], f32, tag="psA",
                                  name="ps_qk")[:, 0, :]
                    for kt in range(KT):
                        nc.tensor.matmul(
                            ps,
                            wqk_sb[:, kt, ct * 128:(ct + 1) * 128],
                            xt_sb[:, kt, :],
                            start=(kt == 0), stop=(kt == KT - 1),
                        )
                    dst = q_sb[ct] if ct < HSC else k_sb[ct - HSC]
                    sl = slice(s0, s0 + 512)
                    # RoPE: dst = ps*cos + swap_half(ps)*sign_sin
                    t1 = tmp.tile([128, 512], f32, tag="ropetmp")
                    nc.vector.tensor_mul(t1[:], ps, cos_sb[:, sl])
                    t2 = tmp.tile([128, 512], f32, tag="ropetmp2")
                    nc.vector.tensor_mul(t2[0:64, :], ps[64:128, :],
                                         sins_sb[0:64, sl])
                    nc.vector.tensor_mul(t2[64:128, :], ps[0:64, :],
                                         sins_sb[64:128, sl])
                    nc.vector.tensor_add(dst[:, sl], t1[:], t2[:])

                # v tiles: natural [s, c] layout
                for sv in range(4):
                    psv = psB.tile([128, 512], f32, tag="psB",
                                   name="psv")[:, 0:HSC * 128]
                    for kt in range(KT):
                        nc.tensor.matmul(
                            psv,
                            xt_sb[:, kt, sv * 128:(sv + 1) * 128],
                            wv_sb[:, kt, :],
                            start=(kt == 0), stop=(kt == KT - 1),
                        )
                    nc.scalar.copy(v_sb[:, e4 * 4 + sv, :], psv)

            # -------- causal attention for batch b --------
            # one AllToAll per head (head-outer loop), so the first
            # collective overlaps the second head's attention and only a
            # half-size collective remains exposed at the batch tail.
            # The very last piece (batch 1, head 1) is further split into
            # two quarter-size collectives by sequence half, so the final
            # o-proj can start on the first half while the second flies.
            pieces = [(0, 0, SL8), (1, 0, SL8)]
            a_ins = [dram.tile([n_cores, 128, p1 - p0], bf16,
                               name=f"a2a_in_{b}_{hh}_{p0}")
                     for hh, p0, p1 in pieces]
            a_outs = [dram.tile([n_cores, 128, p1 - p0], bf16,
                                name=f"a2a_out_{b}_{hh}_{p0}")
                      for hh, p0, p1 in pieces]

            # The whole softmax-normalize chain (rowsum copy, reciprocal,
            # GpSimd partition-broadcast, DVE multiply) runs off the PE, so
            # the PE pipeline is a flat stream of score / attn@V / rowsum
            # matmuls across all chunks and heads, two tile-pairs deep.
            deferred = [None]
            state = {}
            pend = []

            def emit_norm(bb=b, pieces=pieces, a_ins=a_ins, a_outs=a_outs):
                av, bcs, c, h = deferred[0]
                deferred[0] = None
                c0 = bb * S + c * 512
                nc.vector.tensor_mul(
                    attnT_sb[:, h, c0:c0 + 512], av, bcs[:])
                for pc, (hh, p0, p1) in enumerate(pieces):
                    if hh != h:
                        continue
                    for jj in (2 * c, 2 * c + 1):
                        s0 = bb * S + jj * SL8
                        nc.sync.dma_start(
                            a_ins[pc][jj, :, :],
                            attnT_sb[:, h, s0 + p0:s0 + p1],
                        )
                    if c == NCH - 1:
                        nc.gpsimd.collective_compute(
                            "AllToAll",
                            mybir.AluOpType.bypass,
                            replica_groups=[list(range(n_cores))],
                            ins=[a_ins[pc].opt()],
                            outs=[a_outs[pc].opt()],
                        )

            def flush_one():
                # attn@V + rowsum matmuls for the oldest pending tile pair
                # (two pairs behind the score matmuls, so the PE never waits
                # on the ScalarE exp + DVE pair-sum chain)
                ex, spt, pi, c, h, islast = pend.pop(0)
                if pi == 0:
                    state[(c, h)] = [
                        psB.tile([128, 512], f32, tag="psB", name="av"),
                        None,
                    ]
                if pi == 1:
                    state[(c, h)][1] = psR.tile([1, 512], f32, tag="rs",
                                                name="rs")
                av, rs = state[(c, h)]
                if islast:
                    # diagonal pair: columns < 256 are causally zero
                    nc.tensor.matmul(
                        av[:, 256:512], v_sb[:, 2 * pi, h * 128:(h + 1) * 128],
                        ex[:, 0, 256:512], start=False, stop=False)
                    nc.tensor.matmul(
                        av[:, 256:512],
                        v_sb[:, 2 * pi + 1, h * 128:(h + 1) * 128],
                        ex[:, 1, 256:512], start=False, stop=True)
                else:
                    nc.tensor.matmul(
                        av, v_sb[:, 2 * pi, h * 128:(h + 1) * 128],
                        ex[:, 0, :], start=(pi == 0), stop=False)
                    nc.tensor.matmul(
                        av, v_sb[:, 2 * pi + 1, h * 128:(h + 1) * 128],
                        ex[:, 1, :], start=False, stop=False)
                if spt is not None:
                    nc.tensor.matmul(
                        rs, ones_sb[:], spt[:],
                        start=(pi == 1), stop=islast)
                if islast:
                    rs_sb = tmp.tile([1, 512], f32, tag="rs_sb")
                    nc.vector.tensor_copy(rs_sb[:], rs[:])
                    rcp32 = tmp.tile([1, 512], f32, tag="rcp32")
                    nc.vector.reciprocal_approx_fast(rcp32[:], rs_sb[:])
                    bcs = tmp.tile([128, 512], f32, tag="bcs")
                    nc.gpsimd.partition_broadcast(bcs[:], rcp32[:])
                    if deferred[0] is not None:
                        emit_norm()
                    deferred[0] = (av, bcs, c, h)

            for h in range(HSC):
                qh, kh = q_sb[h], k_sb[h]
                for c in range(NCH):
                    qsl = slice(c * 512, (c + 1) * 512)
                    nkt = 4 * c + 4
                    npair = nkt // 2
                    last_pt = [None]
                    for pi in range(npair):
                        kt2 = 2 * pi
                        sc = psA.tile([128, 2, 512], f32, tag="psA", name="sc")
                        nc.tensor.matmul(
                            sc[:, 0, :],
                            kh[:, kt2 * 128:(kt2 + 1) * 128], qh[:, qsl])
                        nc.tensor.matmul(
                            sc[:, 1, :],
                            kh[:, (kt2 + 1) * 128:(kt2 + 2) * 128], qh[:, qsl])
                        if len(pend) == 3:
                            flush_one()
                        ex = expp.tile([128, 2, 512], bf16, tag="ex")
                        if pi == npair - 1:
                            # diagonal pair B: columns < 256 causally zero;
                            # restricted exp + restricted pair mask (the
                            # stale low half is never read)
                            nc.scalar.activation(ex[:, :, 256:512],
                                                 sc[:, :, 256:512],
                                                 Exp, scale=hd_scale)
                            nc.vector.tensor_mul(ex[:, :, 256:512],
                                                 ex[:, :, 256:512],
                                                 pmB_sb[:, :, 256:512])
                        elif pi == npair - 2:
                            nc.scalar.activation(ex[:], sc[:], Exp,
                                                 scale=hd_scale)
                            nc.vector.tensor_mul(ex[:], ex[:], pmA_sb[:])
                        else:
                            nc.scalar.activation(ex[:], sc[:], Exp,
                                                 scale=hd_scale)
                        # denominator tree (DVE): every odd pair sums the
                        # two exp pairs in one wide op, then folds the two
                        # tile-halves into the super-pair sum feeding the
                        # PE rowsum matmul
                        spt = None
                        if pi % 2 == 0:
                            last_pt[0] = ex
                        else:
                            lex = last_pt[0]
                            tp = ptp.tile([128, 2, 512], bf16, tag="tp")
                            if pi == npair - 1:
                                nc.vector.tensor_copy(tp[:, :, 0:256],
                                                      lex[:, :, 0:256])
                                nc.vector.tensor_add(tp[:, :, 256:512],
                                                     lex[:, :, 256:512],
                                                     ex[:, :, 256:512])
                            else:
                                nc.vector.tensor_add(tp[:], lex[:], ex[:])
                            spt = sptp.tile([128, 512], bf16, tag="spt")
                            nc.vector.tensor_add(spt[:], tp[:, 0, :],
                                                 tp[:, 1, :])
                        pend.append((ex, spt, pi, c, h, pi == npair - 1))
            while pend:
                flush_one()
            # last chunk/head: emit its normalize (and its collective) now
            emit_norm()
            a2a_outs.append((pieces, a_outs))

        # -------- output projection for this core's row slices --------
        # core's out rows: [0:SL8] = batch-0 slice, [SL8:2*SL8] = batch-1
        # batch-outer so the batch-1 gather (which waits on the second
        # AllToAll) never blocks batch-0's W_o loads in the Sync DMA queue.
        # Batch 1 walks ec in reverse: ec3/ec2 W_o tiles are still resident
        # from batch 0 (wop bufs=3), so its first matmuls only wait on the
        # gather; ec1/ec0 reload in the shadow of ec3/ec2 compute.
        STB = max(1, SL8 // 128)
        PS = min(128, SL8)
        wo_tiles = {}

        def load_wo(ec, name):
            wo_sb = wop.tile([128, H, 512], bf16, tag="wo", name=name)
            nc.sync.dma_start(wo_sb[:], wo_rs[:, :, ec * 512:(ec + 1) * 512])
            wo_tiles[ec] = wo_sb

        # accumulate first-head (even) rows first so the second head's
        # gather gets extra slack
        ht_order = [r * HSC for r in range(n_cores)] + \
                   [r * HSC + 1 for r in range(n_cores)]

        def po_group(b, ec, st, atn_sb):
            po = psB.tile([128, 512], f32, tag="psB", name="po")[:PS]
            wo_sb = wo_tiles[ec]
            for hi, ht in enumerate(ht_order):
                nc.tensor.matmul(
                    po,
                    atn_sb[:, ht // HSC, ht % HSC, st * 128:st * 128 + PS],
                    wo_sb[:, ht, :],
                    start=(hi == 0), stop=(hi == H - 1),
                )
            ot = tmp.tile([128, 512], f32, tag="ot")
            nc.scalar.copy(ot[:PS, :], po)
            r0 = b * SL8 + st * 128
            nc.sync.dma_start(out[r0:r0 + PS, ec * 512:(ec + 1) * 512],
                              ot[:PS, :])

        def gather(b, pc, atn_sb):
            hh, p0, p1 = a2a_outs[b][0][pc]
            nc.sync.dma_start(atn_sb[:, :, hh, p0:p1],
                              a2a_outs[b][1][pc].rearrange("r p s -> p r s"))

        # batch 0: plain ec-major order
        atn0 = stream.tile([128, n_cores, HSC, SL8], bf16, tag="stream",
                           name="atn_0")
        gather(0, 0, atn0)
        gather(0, 1, atn0)
        for ec in range(ECH):
            load_wo(ec, f"wo_b0_{ec}")
            for st in range(STB):
                po_group(0, ec, st, atn0)
        # batch 1: ec3/ec2 W_o still resident from batch 0; st=0 rows only
        # need the first half-gather of head 1, so they run while the last
        # quarter-size collective is still in flight
        atn1 = stream.tile([128, n_cores, HSC, SL8], bf16, tag="stream",
                           name="atn_1")
        gather(1, 0, atn1)           # head 0
        load_wo(1, "wo_b1_1")
        gather(1, 1, atn1)           # head 1
        for ec in (3, 2):
            for st in range(STB):
                po_group(1, ec, st, atn1)
        load_wo(0, "wo_b1_0")
        for ec in (1, 0):
            for st in range(STB):
                po_group(1, ec, st, atn1)

    nc.finalize()
    return nc


# ---------------------------------------------------------------- host code
def make_tables(S):
    half = HD // 2
    inv_freq = (1.0 / (10000.0 ** (np.arange(half, dtype=np.float32) / half)))
    pos = np.arange(S, dtype=np.float32)
    freqs = pos[:, None] * inv_freq[None, :]          # [S, half]
    cos = np.cos(freqs).astype(np.float32)            # [S, half]
    sin = np.sin(freqs).astype(np.float32)
    cosT = np.concatenate([cos, cos], axis=1).T       # [HD, S]
    # sign-folded sin: rows 0..63 get -sin, rows 64..127 get +sin
    sinsT = np.concatenate([-sin, sin], axis=1).T     # [HD, S]
    return np.ascontiguousarray(cosT), np.ascontiguousarray(sinsT)


def make_mask():
    j = np.arange(896)[None, :]
    k = np.arange(128)[:, None]
    return ((j - 384) >= k).astype(np.float32)        # [128, 896]


def make_pair_masks():
    m = make_mask()
    pmA = np.concatenate([m[:, 384:896], m[:, 256:768]], axis=1)  # [128,1024]
    pmB = np.concatenate([m[:, 128:640], m[:, 0:512]], axis=1)
    return pmA, pmB


def prepare_in_maps(x, W_qkv, W_o, S, D):
    import ml_dtypes
    bf16 = ml_dtypes.bfloat16

    S2 = B * S
    xT = np.ascontiguousarray(
        x.reshape(S2, D).T.astype(np.float32)).astype(bf16)
    cosT, sinsT = make_tables(S)
    pmA, pmB = make_pair_masks()
    ones = np.ones((128, 1), bf16)
    wo_bf16 = W_o.astype(bf16)

    qw = W_qkv[:, 0 * H * HD:1 * H * HD]
    kw = W_qkv[:, 1 * H * HD:2 * H * HD]
    vw = W_qkv[:, 2 * H * HD:3 * H * HD]

    in_maps = []
    for c in range(N_CORES):
        h0 = c * HEADS_PER_CORE
        cols = slice(h0 * HD, (h0 + HEADS_PER_CORE) * HD)
        wqk_c = np.ascontiguousarray(
            np.concatenate([qw[:, cols], kw[:, cols]], axis=1)).astype(bf16)
        wv_c = np.ascontiguousarray(vw[:, cols]).astype(bf16)
        in_maps.append({
            "xt": xT, "wqk": wqk_c, "wv": wv_c, "wo": wo_bf16,
            "cost": cosT.astype(bf16), "sins": sinsT.astype(bf16),
            "pmA": pmA.astype(bf16), "pmB": pmB.astype(bf16), "ones": ones,
        })
    return in_maps


_NC_CACHE = {}


def run(x, W_qkv, W_o, S, D, trace=False, trace_kwargs=None):
    from concourse.bass_utils import run_bass_kernel_spmd

    key = (S, D)
    if key not in _NC_CACHE:
        _NC_CACHE[key] = build_nc(S=S, D=D)
    nc = _NC_CACHE[key]
    in_maps = prepare_in_maps(x, W_qkv, W_o, S, D)
    res = run_bass_kernel_spmd(
        nc, in_maps, core_ids=list(range(N_CORES)),
        trace=trace, **(trace_kwargs or {}),
    )
    SL8 = S // N_CORES
    full = np.empty((B, S, D), np.float32)
    for c in range(N_CORES):
        o = res.results[c]["out"]
        full[0, c * SL8:(c + 1) * SL8] = o[:SL8]
        full[1, c * SL8:(c + 1) * SL8] = o[SL8:]
    return full, res


def kernel(x, W_qkv, W_o):
    x = np.asarray(x)
    W_qkv = np.asarray(W_qkv)
    W_o = np.asarray(W_o)
    S, D = x.shape[1], x.shape[2]
    out, _ = run(x, W_qkv, W_o, S, D, trace=False)
    return out.astype(np.float32)


# revision 51
# speedup vs baseline: 1.0045x; 1.0045x over previous
"""Trainium2 Bass kernel for nn_MultiHeadAttention (B=2, S=2048, D=2048, H=16, HD=128).

Sharding: tensor-parallel across heads; each of the 8 cores owns 2 heads.

Per core:
  - QKV projection for its heads (bf16 matmuls, fp32 PSUM accumulation,
    K-contiguous, N=512); RoPE on DVE with host-precomputed cos /
    sign-folded sin tables (bf16).
  - Causal attention in transposed layout scores^T[k, q], run as one flat
    software pipeline across all chunks and heads: the PE issues score
    matmul pairs three tile-pairs ahead of the deferred attn@V + rowsum
    matmuls, ScalarE exps tile-pairs 1024 wide straight out of PSUM, and
    the DVE folds exp pairs into super-pair sums that a single ones-vector
    matmul per two pairs accumulates into the softmax denominator (fp32
    PSUM).  The whole normalize chain (rowsum copy, fast approx
    reciprocal, GpSimd partition-broadcast, DVE multiply) runs off the PE.
    The diagonal tile-pair uses a column-restricted exp / mask / attn@V
    (its low half is causally zero); causal masks are two host-precomputed
    1024-wide pair masks.
  - One AllToAll per (batch, head) redistributes head-outputs into
    sequence-slices (bf16): head 0's collective overlaps head 1's
    attention, so only a half-size collective is exposed at each batch
    tail, where it is covered by the previous batch's output projection.
    Shard layout [dest, partition, seq] makes the post-collective gather
    DMA read 1KB-contiguous runs.
  - Output projection W_o in bf16 (16-matmul fp32 accumulation groups);
    batch 1 walks the e-chunks in reverse so the W_o tiles still resident
    from batch 0 cover its first matmuls while the rest reload.
Host gathers the full output by concatenating the 8 row-slices.
"""

import numpy as np

B = 2
H = 16
HD = 128
N_CORES = 8
HEADS_PER_CORE = H // N_CORES


# ---------------------------------------------------------------- device code
def build_nc(S=2048, D=2048, n_cores=N_CORES):
    from contextlib import ExitStack

    import concourse.mybir as mybir
    import concourse.tile as tile
    from concourse import bacc

    f32 = mybir.dt.float32
    bf16 = mybir.dt.bfloat16
    Exp = mybir.ActivationFunctionType.Exp

    KT = D // 128            # contraction tiles for projections
    S2 = B * S               # total rows of x
    NCH = S // 512           # 512-wide q-chunks per batch
    SLICE = S2 // n_cores    # output rows per core
    ECH = D // 512           # 512-wide e-chunks of the output dim
    HSC = HEADS_PER_CORE
    SL8 = S // n_cores
    hd_scale = float(HD) ** -0.5

    nc = bacc.Bacc("TRN2", target_bir_lowering=False, debug=False,
                   num_devices=n_cores)

    xt = nc.dram_tensor("xt", [D, S2], bf16, kind="ExternalInput").ap()
    wqk = nc.dram_tensor("wqk", [D, 2 * HSC * 128], bf16, kind="ExternalInput").ap()
    wv = nc.dram_tensor("wv", [D, HSC * 128], bf16, kind="ExternalInput").ap()
    wo = nc.dram_tensor("wo", [H * HD, D], bf16, kind="ExternalInput").ap()
    cost = nc.dram_tensor("cost", [128, S], bf16, kind="ExternalInput").ap()
    sins = nc.dram_tensor("sins", [128, S], bf16, kind="ExternalInput").ap()
    pmA = nc.dram_tensor("pmA", [128, 2 * 512], bf16, kind="ExternalInput").ap()
    pmB = nc.dram_tensor("pmB", [128, 2 * 512], bf16, kind="ExternalInput").ap()
    ones = nc.dram_tensor("ones", [128, 1], bf16, kind="ExternalInput").ap()
    out = nc.dram_tensor("out", [SLICE, D], f32, kind="ExternalOutput").ap()

    xt_rs = xt.rearrange("(kt p) s -> p kt s", p=128)
    wqk_rs = wqk.rearrange("(kt p) m -> p kt m", p=128)
    wv_rs = wv.rearrange("(kt p) m -> p kt m", p=128)
    wo_rs = wo.rearrange("(ht p) e -> p ht e", p=128)

    with tile.TileContext(nc) as tc, ExitStack() as ctx:
        const = ctx.enter_context(tc.tile_pool(name="const", bufs=1))
        stream = ctx.enter_context(tc.tile_pool(name="stream", bufs=2))
        wop = ctx.enter_context(tc.tile_pool(name="wop", bufs=3))
        qkvp = ctx.enter_context(tc.tile_pool(name="qkvp", bufs=1))
        expp = ctx.enter_context(tc.tile_pool(name="expp", bufs=6))
        ptp = ctx.enter_context(tc.tile_pool(name="ptp", bufs=3))
        sptp = ctx.enter_context(tc.tile_pool(name="sptp", bufs=3))
        tmp = ctx.enter_context(tc.tile_pool(name="tmp", bufs=2))
        # PSUM budget (8 banks): psA 2 slots x 2 banks (qk-proj ps / score
        # pairs), psB 3 slots x 1 bank (v-proj ps / attn@V acc / o-proj ps),
        # psR 1 bank (rowsum).
        psA = ctx.enter_context(tc.tile_pool(name="psA", bufs=2, space="PSUM"))
        psB = ctx.enter_context(tc.tile_pool(name="psB", bufs=3, space="PSUM"))
        psR = ctx.enter_context(tc.tile_pool(name="psR", bufs=1, space="PSUM"))
        dram = ctx.enter_context(tc.tile_pool(name="dram", bufs=1, space="DRAM"))

        # resident constants; wqk + the first x chunk first so the PE can
        # start as early as possible
        # split the first DMAs so they spread across queues and the first
        # matmul group can start as early as possible
        wqk_sb = const.tile([128, KT, 2 * HSC * 128], bf16)
        for ct in range(2 * HSC):
            nc.sync.dma_start(wqk_sb[:, :, ct * 128:(ct + 1) * 128],
                              wqk_rs[:, :, ct * 128:(ct + 1) * 128])
        xt0_sb = stream.tile([128, KT, 512], bf16, tag="stream", name="xt0")
        nc.sync.dma_start(xt0_sb[:, 0:KT // 2, :], xt_rs[:, 0:KT // 2, 0:512])
        nc.sync.dma_start(xt0_sb[:, KT // 2:, :], xt_rs[:, KT // 2:, 0:512])
        xt1_sb = stream.tile([128, KT, 512], bf16, tag="stream", name="xt1")
        nc.sync.dma_start(xt1_sb[:, 0:KT // 2, :],
                          xt_rs[:, 0:KT // 2, 512:1024])
        nc.sync.dma_start(xt1_sb[:, KT // 2:, :],
                          xt_rs[:, KT // 2:, 512:1024])
        cos_sb = const.tile([128, S], bf16)
        nc.sync.dma_start(cos_sb[:], cost[:])
        sins_sb = const.tile([128, S], bf16)
        nc.sync.dma_start(sins_sb[:], sins[:])
        wv_sb = const.tile([128, KT, HSC * 128], bf16)
        nc.sync.dma_start(wv_sb[:], wv_rs[:])
        pmA_sb = const.tile([128, 2, 512], bf16)
        nc.sync.dma_start(pmA_sb[:], pmA.rearrange("p (t q) -> p t q", t=2))
        pmB_sb = const.tile([128, 2, 512], bf16)
        nc.sync.dma_start(pmB_sb[:], pmB.rearrange("p (t q) -> p t q", t=2))
        ones_sb = const.tile([128, 1], bf16)
        nc.sync.dma_start(ones_sb[:], ones[:])

        # normalized attention output, transposed: [d, head, b*S+s] (bf16)
        attnT_sb = const.tile([128, HSC, S2], bf16)

        # the diagonal tile-pair uses a column-restricted exp, so its exp
        # buffers are partially stale; zero them once so stale bits can
        # never be NaN/Inf patterns (the pair mask multiplies them to 0)
        for i in range(6):
            exz = expp.tile([128, 2, 512], bf16, tag="ex", name=f"exz{i}")
            nc.any.memset(exz[:], 0.0)

        a2a_outs = []
        for b in range(B):
            # -------- QKV projection for batch b (heads of this core) -----
            q_sb = [qkvp.tile([128, S], bf16, tag=f"q{h}", name=f"q{h}_{b}")
                    for h in range(HSC)]
            k_sb = [qkvp.tile([128, S], bf16, tag=f"k{h}", name=f"k{h}_{b}")
                    for h in range(HSC)]
            v_sb = qkvp.tile([128, S // 128, HSC * 128], bf16, tag="v")

            for e4 in range(S // 512):
                s0 = e4 * 512
                if b == 0 and e4 == 0:
                    xt_sb = xt0_sb
                elif b == 0 and e4 == 1:
                    xt_sb = xt1_sb
                else:
                    xt_sb = stream.tile([128, KT, 512], bf16, tag="stream")
                    nc.sync.dma_start(
                        xt_sb[:], xt_rs[:, :, b * S + s0:b * S + s0 + 512])

                # q/k tiles: out^T layout [c, s], N=512
                for ct in range(2 * HSC):
                    ps = psA.tile([128, 2, 512], f32, tag="psA",
                                  name="ps_qk")[:, 0, :]
                    for kt in range(KT):
                        nc.tensor.matmul(
                            ps,
                            wqk_sb[:, kt, ct * 128:(ct + 1) * 128],
                            xt_sb[:, kt, :],
                            start=(kt == 0), stop=(kt == KT - 1),
                        )
                    dst = q_sb[ct] if ct < HSC else k_sb[ct - HSC]
                    sl = slice(s0, s0 + 512)
                    # RoPE: dst = ps*cos + swap_half(ps)*sign_sin.  The
                    # rotate-half runs through a ScalarE partition-swap copy
                    # (bf16), so the DVE applies the sign-folded sin in one
                    # full-width 2x-mode multiply and the PSUM bank is
                    # released for the PE's next group after just two fast
                    # ScalarE copies + one DVE read.
                    psw = tmp.tile([128, 512], bf16, tag="psw")
                    nc.scalar.copy(psw[0:64, :], ps[64:128, :])
                    nc.scalar.copy(psw[64:128, :], ps[0:64, :])
                    t1 = tmp.tile([128, 512], f32, tag="ropetmp")
                    nc.vector.tensor_mul(t1[:], ps, cos_sb[:, sl])
                    t2 = tmp.tile([128, 512], bf16, tag="ropetmp2")
                    nc.vector.tensor_mul(t2[:], psw[:], sins_sb[:, sl])
                    nc.vector.tensor_add(dst[:, sl], t1[:], t2[:])

                # v tiles: natural [s, c] layout
                for sv in range(4):
                    psv = psB.tile([128, 512], f32, tag="psB",
                                   name="psv")[:, 0:HSC * 128]
                    for kt in range(KT):
                        nc.tensor.matmul(
                            psv,
                            xt_sb[:, kt, sv * 128:(sv + 1) * 128],
                            wv_sb[:, kt, :],
                            start=(kt == 0), stop=(kt == KT - 1),
                        )
                    nc.scalar.copy(v_sb[:, e4 * 4 + sv, :], psv)

            # -------- causal attention for batch b --------
            # one AllToAll per head (head-outer loop), so the first
            # collective overlaps the second head's attention and only a
            # half-size collective remains exposed at the batch tail.
            # The very last piece (batch 1, head 1) is further split into
            # two quarter-size collectives by sequence half, so the final
            # o-proj can start on the first half while the second flies.
            pieces = [(0, 0, SL8), (1, 0, SL8)]
            a_ins = [dram.tile([n_cores, 128, p1 - p0], bf16,
                               name=f"a2a_in_{b}_{hh}_{p0}")
                     for hh, p0, p1 in pieces]
            a_outs = [dram.tile([n_cores, 128, p1 - p0], bf16,
                                name=f"a2a_out_{b}_{hh}_{p0}")
                      for hh, p0, p1 in pieces]

            # The whole softmax-normalize chain (rowsum copy, reciprocal,
            # GpSimd partition-broadcast, DVE multiply) runs off the PE, so
            # the PE pipeline is a flat stream of score / attn@V / rowsum
            # matmuls across all chunks and heads, two tile-pairs deep.
            deferred = [None]
            state = {}
            pend = []

            def emit_norm(bb=b, pieces=pieces, a_ins=a_ins, a_outs=a_outs):
                av, bcs, c, h = deferred[0]
                deferred[0] = None
                c0 = bb * S + c * 512
                nc.vector.tensor_mul(
                    attnT_sb[:, h, c0:c0 + 512], av, bcs[:])
                for pc, (hh, p0, p1) in enumerate(pieces):
                    if hh != h:
                        continue
                    for jj in (2 * c, 2 * c + 1):
                        s0 = bb * S + jj * SL8
                        nc.sync.dma_start(
                            a_ins[pc][jj, :, :],
                            attnT_sb[:, h, s0 + p0:s0 + p1],
                        )
                    if c == NCH - 1:
                        nc.gpsimd.collective_compute(
                            "AllToAll",
                            mybir.AluOpType.bypass,
                            replica_groups=[list(range(n_cores))],
                            ins=[a_ins[pc].opt()],
                            outs=[a_outs[pc].opt()],
                        )

            def flush_one():
                # attn@V + rowsum matmuls for the oldest pending tile pair
                # (two pairs behind the score matmuls, so the PE never waits
                # on the ScalarE exp + DVE pair-sum chain)
                ex, spt, pi, c, h, islast = pend.pop(0)
                if pi == 0:
                    state[(c, h)] = [
                        psB.tile([128, 512], f32, tag="psB", name="av"),
                        None,
                    ]
                if pi == 1:
                    state[(c, h)][1] = psR.tile([1, 512], f32, tag="rs",
                                                name="rs")
                av, rs = state[(c, h)]
                if islast:
                    # diagonal pair: columns < 256 are causally zero
                    nc.tensor.matmul(
                        av[:, 256:512], v_sb[:, 2 * pi, h * 128:(h + 1) * 128],
                        ex[:, 0, 256:512], start=False, stop=False)
                    nc.tensor.matmul(
                        av[:, 256:512],
                        v_sb[:, 2 * pi + 1, h * 128:(h + 1) * 128],
                        ex[:, 1, 256:512], start=False, stop=True)
                else:
                    nc.tensor.matmul(
                        av, v_sb[:, 2 * pi, h * 128:(h + 1) * 128],
                        ex[:, 0, :], start=(pi == 0), stop=False)
                    nc.tensor.matmul(
                        av, v_sb[:, 2 * pi + 1, h * 128:(h + 1) * 128],
                        ex[:, 1, :], start=False, stop=False)
                if spt is not None:
                    nc.tensor.matmul(
                        rs, ones_sb[:], spt[:],
                        start=(pi == 1), stop=islast)
                if islast:
                    rs_sb = tmp.tile([1, 512], f32, tag="rs_sb")
                    nc.vector.tensor_copy(rs_sb[:], rs[:])
                    rcp32 = tmp.tile([1, 512], f32, tag="rcp32")
                    nc.vector.reciprocal_approx_fast(rcp32[:], rs_sb[:])
                    bcs = tmp.tile([128, 512], f32, tag="bcs")
                    nc.gpsimd.partition_broadcast(bcs[:], rcp32[:])
                    if deferred[0] is not None:
                        emit_norm()
                    deferred[0] = (av, bcs, c, h)

            for h in range(HSC):
                qh, kh = q_sb[h], k_sb[h]
                for c in range(NCH):
                    qsl = slice(c * 512, (c + 1) * 512)
                    nkt = 4 * c + 4
                    npair = nkt // 2
                    last_pt = [None]
                    for pi in range(npair):
                        kt2 = 2 * pi
                        sc = psA.tile([128, 2, 512], f32, tag="psA", name="sc")
                        nc.tensor.matmul(
                            sc[:, 0, :],
                            kh[:, kt2 * 128:(kt2 + 1) * 128], qh[:, qsl])
                        nc.tensor.matmul(
                            sc[:, 1, :],
                            kh[:, (kt2 + 1) * 128:(kt2 + 2) * 128], qh[:, qsl])
                        if len(pend) == 3:
                            flush_one()
                        ex = expp.tile([128, 2, 512], bf16, tag="ex")
                        if pi == npair - 1:
                            # diagonal pair B: columns < 256 causally zero;
                            # restricted exp + restricted pair mask (the
                            # stale low half is never read)
                            nc.scalar.activation(ex[:, :, 256:512],
                                                 sc[:, :, 256:512],
                                                 Exp, scale=hd_scale)
                            nc.vector.tensor_mul(ex[:, :, 256:512],
                                                 ex[:, :, 256:512],
                                                 pmB_sb[:, :, 256:512])
                        elif pi == npair - 2:
                            nc.scalar.activation(ex[:], sc[:], Exp,
                                                 scale=hd_scale)
                            nc.vector.tensor_mul(ex[:], ex[:], pmA_sb[:])
                        else:
                            nc.scalar.activation(ex[:], sc[:], Exp,
                                                 scale=hd_scale)
                        # denominator tree (DVE): every odd pair sums the
                        # two exp pairs in one wide op, then folds the two
                        # tile-halves into the super-pair sum feeding the
                        # PE rowsum matmul
                        spt = None
                        if pi % 2 == 0:
                            last_pt[0] = ex
                        else:
                            lex = last_pt[0]
                            tp = ptp.tile([128, 2, 512], bf16, tag="tp")
                            if pi == npair - 1:
                                nc.vector.tensor_copy(tp[:, :, 0:256],
                                                      lex[:, :, 0:256])
                                nc.vector.tensor_add(tp[:, :, 256:512],
                                                     lex[:, :, 256:512],
                                                     ex[:, :, 256:512])
                            else:
                                nc.vector.tensor_add(tp[:], lex[:], ex[:])
                            spt = sptp.tile([128, 512], bf16, tag="spt")
                            nc.vector.tensor_add(spt[:], tp[:, 0, :],
                                                 tp[:, 1, :])
                        pend.append((ex, spt, pi, c, h, pi == npair - 1))
            while pend:
                flush_one()
            # last chunk/head: emit its normalize (and its collective) now
            emit_norm()
            a2a_outs.append((pieces, a_outs))

        # -------- output projection for this core's row slices --------
        # core's out rows: [0:SL8] = batch-0 slice, [SL8:2*SL8] = batch-1
        # batch-outer so the batch-1 gather (which waits on the second
        # AllToAll) never blocks batch-0's W_o loads in the Sync DMA queue.
        # Batch 1 walks ec in reverse: ec3/ec2 W_o tiles are still resident
        # from batch 0 (wop bufs=3), so its first matmuls only wait on the
        # gather; ec1/ec0 reload in the shadow of ec3/ec2 compute.
        STB = max(1, SL8 // 128)
        PS = min(128, SL8)
        wo_tiles = {}

        def load_wo(ec, name):
            wo_sb = wop.tile([128, H, 512], bf16, tag="wo", name=name)
            nc.sync.dma_start(wo_sb[:], wo_rs[:, :, ec * 512:(ec + 1) * 512])
            wo_tiles[ec] = wo_sb

        # accumulate first-head (even) rows first so the second head's
        # gather gets extra slack
        ht_order = [r * HSC for r in range(n_cores)] + \
                   [r * HSC + 1 for r in range(n_cores)]

        def po_group(b, ec, st, atn_sb):
            po = psB.tile([128, 512], f32, tag="psB", name="po")[:PS]
            wo_sb = wo_tiles[ec]
            for hi, ht in enumerate(ht_order):
                nc.tensor.matmul(
                    po,
                    atn_sb[:, ht // HSC, ht % HSC, st * 128:st * 128 + PS],
                    wo_sb[:, ht, :],
                    start=(hi == 0), stop=(hi == H - 1),
                )
            ot = tmp.tile([128, 512], f32, tag="ot")
            nc.scalar.copy(ot[:PS, :], po)
            r0 = b * SL8 + st * 128
            nc.sync.dma_start(out[r0:r0 + PS, ec * 512:(ec + 1) * 512],
                              ot[:PS, :])

        def gather(b, pc, atn_sb):
            hh, p0, p1 = a2a_outs[b][0][pc]
            nc.sync.dma_start(atn_sb[:, :, hh, p0:p1],
                              a2a_outs[b][1][pc].rearrange("r p s -> p r s"))

        # batch 0: plain ec-major order
        atn0 = stream.tile([128, n_cores, HSC, SL8], bf16, tag="stream",
                           name="atn_0")
        gather(0, 0, atn0)
        gather(0, 1, atn0)
        for ec in range(ECH):
            load_wo(ec, f"wo_b0_{ec}")
            for st in range(STB):
                po_group(0, ec, st, atn0)
        # batch 1: ec3/ec2 W_o still resident from batch 0; st=0 rows only
        # need the first half-gather of head 1, so they run while the last
        # quarter-size collective is still in flight
        atn1 = stream.tile([128, n_cores, HSC, SL8], bf16, tag="stream",
                           name="atn_1")
        gather(1, 0, atn1)           # head 0
        load_wo(1, "wo_b1_1")
        gather(1, 1, atn1)           # head 1
        for ec in (3, 2):
            for st in range(STB):
                po_group(1, ec, st, atn1)
        load_wo(0, "wo_b1_0")
        for ec in (1, 0):
            for st in range(STB):
                po_group(1, ec, st, atn1)

    nc.finalize()
    return nc


# ---------------------------------------------------------------- host code
def make_tables(S):
    half = HD // 2
    inv_freq = (1.0 / (10000.0 ** (np.arange(half, dtype=np.float32) / half)))
    pos = np.arange(S, dtype=np.float32)
    freqs = pos[:, None] * inv_freq[None, :]          # [S, half]
    cos = np.cos(freqs).astype(np.float32)            # [S, half]
    sin = np.sin(freqs).astype(np.float32)
    cosT = np.concatenate([cos, cos], axis=1).T       # [HD, S]
    # sign-folded sin: rows 0..63 get -sin, rows 64..127 get +sin
    sinsT = np.concatenate([-sin, sin], axis=1).T     # [HD, S]
    return np.ascontiguousarray(cosT), np.ascontiguousarray(sinsT)


def make_mask():
    j = np.arange(896)[None, :]
    k = np.arange(128)[:, None]
    return ((j - 384) >= k).astype(np.float32)        # [128, 896]


def make_pair_masks():
    m = make_mask()
    pmA = np.concatenate([m[:, 384:896], m[:, 256:768]], axis=1)  # [128,1024]
    pmB = np.concatenate([m[:, 128:640], m[:, 0:512]], axis=1)
    return pmA, pmB


def prepare_in_maps(x, W_qkv, W_o, S, D):
    import ml_dtypes
    bf16 = ml_dtypes.bfloat16

    S2 = B * S
    xT = np.ascontiguousarray(
        x.reshape(S2, D).T.astype(np.float32)).astype(bf16)
    cosT, sinsT = make_tables(S)
    pmA, pmB = make_pair_masks()
    ones = np.ones((128, 1), bf16)
    wo_bf16 = W_o.astype(bf16)

    qw = W_qkv[:, 0 * H * HD:1 * H * HD]
    kw = W_qkv[:, 1 * H * HD:2 * H * HD]
    vw = W_qkv[:, 2 * H * HD:3 * H * HD]

    in_maps = []
    for c in range(N_CORES):
        h0 = c * HEADS_PER_CORE
        cols = slice(h0 * HD, (h0 + HEADS_PER_CORE) * HD)
        wqk_c = np.ascontiguousarray(
            np.concatenate([qw[:, cols], kw[:, cols]], axis=1)).astype(bf16)
        wv_c = np.ascontiguousarray(vw[:, cols]).astype(bf16)
        in_maps.append({
            "xt": xT, "wqk": wqk_c, "wv": wv_c, "wo": wo_bf16,
            "cost": cosT.astype(bf16), "sins": sinsT.astype(bf16),
            "pmA": pmA.astype(bf16), "pmB": pmB.astype(bf16), "ones": ones,
        })
    return in_maps


_NC_CACHE = {}


def run(x, W_qkv, W_o, S, D, trace=False, trace_kwargs=None):
    from concourse.bass_utils import run_bass_kernel_spmd

    key = (S, D)
    if key not in _NC_CACHE:
        _NC_CACHE[key] = build_nc(S=S, D=D)
    nc = _NC_CACHE[key]
    in_maps = prepare_in_maps(x, W_qkv, W_o, S, D)
    res = run_bass_kernel_spmd(
        nc, in_maps, core_ids=list(range(N_CORES)),
        trace=trace, **(trace_kwargs or {}),
    )
    SL8 = S // N_CORES
    full = np.empty((B, S, D), np.float32)
    for c in range(N_CORES):
        o = res.results[c]["out"]
        full[0, c * SL8:(c + 1) * SL8] = o[:SL8]
        full[1, c * SL8:(c + 1) * SL8] = o[SL8:]
    return full, res


def kernel(x, W_qkv, W_o):
    x = np.asarray(x)
    W_qkv = np.asarray(W_qkv)
    W_o = np.asarray(W_o)
    S, D = x.shape[1], x.shape[2]
    out, _ = run(x, W_qkv, W_o, S, D, trace=False)
    return out.astype(np.float32)


# revision 54
# speedup vs baseline: 1.0202x; 1.0156x over previous
"""Trainium2 Bass kernel for nn_MultiHeadAttention (B=2, S=2048, D=2048, H=16, HD=128).

Sharding: tensor-parallel across heads; each of the 8 cores owns 2 heads.

Per core:
  - QKV projection for its heads (bf16 matmuls, fp32 PSUM accumulation,
    K-contiguous, N=512); RoPE on DVE with host-precomputed cos /
    sign-folded sin tables (bf16).
  - Causal attention in transposed layout scores^T[k, q], run as one flat
    software pipeline across all chunks and heads: the PE issues score
    matmul pairs three tile-pairs ahead of the deferred attn@V + rowsum
    matmuls, ScalarE exps tile-pairs 1024 wide straight out of PSUM, and
    the DVE folds exp pairs into super-pair sums that a single ones-vector
    matmul per two pairs accumulates into the softmax denominator (fp32
    PSUM).  The whole normalize chain (rowsum copy, fast approx
    reciprocal, GpSimd partition-broadcast, DVE multiply) runs off the PE.
    The diagonal tile-pair uses a column-restricted exp / mask / attn@V
    (its low half is causally zero); causal masks are two host-precomputed
    1024-wide pair masks.
  - One AllToAll per (batch, head) redistributes head-outputs into
    sequence-slices (bf16): head 0's collective overlaps head 1's
    attention, so only a half-size collective is exposed at each batch
    tail, where it is covered by the previous batch's output projection.
    Shard layout [dest, partition, seq] makes the post-collective gather
    DMA read 1KB-contiguous runs.
  - Output projection W_o in bf16 (16-matmul fp32 accumulation groups);
    batch 1 walks the e-chunks in reverse so the W_o tiles still resident
    from batch 0 cover its first matmuls while the rest reload.
Host gathers the full output by concatenating the 8 row-slices.
"""

import numpy as np

B = 2
H = 16
HD = 128
N_CORES = 8
HEADS_PER_CORE = H // N_CORES


# ---------------------------------------------------------------- device code
def build_nc(S=2048, D=2048, n_cores=N_CORES):
    from contextlib import ExitStack

    import concourse.mybir as mybir
    import concourse.tile as tile
    from concourse import bacc

    f32 = mybir.dt.float32
    bf16 = mybir.dt.bfloat16
    Exp = mybir.ActivationFunctionType.Exp

    KT = D // 128            # contraction tiles for projections
    S2 = B * S               # total rows of x
    NCH = S // 512           # 512-wide q-chunks per batch
    SLICE = S2 // n_cores    # output rows per core
    ECH = D // 512           # 512-wide e-chunks of the output dim
    HSC = HEADS_PER_CORE
    SL8 = S // n_cores
    hd_scale = float(HD) ** -0.5

    nc = bacc.Bacc("TRN2", target_bir_lowering=False, debug=False,
                   num_devices=n_cores)

    xt = nc.dram_tensor("xt", [D, S2], bf16, kind="ExternalInput").ap()
    wqk = nc.dram_tensor("wqk", [D, 2 * HSC * 128], bf16, kind="ExternalInput").ap()
    wv = nc.dram_tensor("wv", [D, HSC * 128], bf16, kind="ExternalInput").ap()
    wo = nc.dram_tensor("wo", [H * HD, D], bf16, kind="ExternalInput").ap()
    cost = nc.dram_tensor("cost", [128, S], bf16, kind="ExternalInput").ap()
    sins = nc.dram_tensor("sins", [128, S], bf16, kind="ExternalInput").ap()
    pmA = nc.dram_tensor("pmA", [128, 2 * 512], bf16, kind="ExternalInput").ap()
    pmB = nc.dram_tensor("pmB", [128, 2 * 512], bf16, kind="ExternalInput").ap()
    ones = nc.dram_tensor("ones", [128, 1], bf16, kind="ExternalInput").ap()
    out = nc.dram_tensor("out", [SLICE, D], f32, kind="ExternalOutput").ap()

    xt_rs = xt.rearrange("(kt p) s -> p kt s", p=128)
    wqk_rs = wqk.rearrange("(kt p) m -> p kt m", p=128)
    wv_rs = wv.rearrange("(kt p) m -> p kt m", p=128)
    wo_rs = wo.rearrange("(ht p) e -> p ht e", p=128)

    with tile.TileContext(nc) as tc, ExitStack() as ctx:
        const = ctx.enter_context(tc.tile_pool(name="const", bufs=1))
        stream = ctx.enter_context(tc.tile_pool(name="stream", bufs=2))
        wop = ctx.enter_context(tc.tile_pool(name="wop", bufs=3))
        qkvp = ctx.enter_context(tc.tile_pool(name="qkvp", bufs=1))
        expp = ctx.enter_context(tc.tile_pool(name="expp", bufs=6))
        ptp = ctx.enter_context(tc.tile_pool(name="ptp", bufs=3))
        sptp = ctx.enter_context(tc.tile_pool(name="sptp", bufs=3))
        tmp = ctx.enter_context(tc.tile_pool(name="tmp", bufs=2))
        # PSUM budget (8 banks): psA 2 slots x 2 banks (qk-proj ps / score
        # pairs), psB 3 slots x 1 bank (v-proj ps / attn@V acc / o-proj ps),
        # psR 1 bank (rowsum).
        psA = ctx.enter_context(tc.tile_pool(name="psA", bufs=2, space="PSUM"))
        psB = ctx.enter_context(tc.tile_pool(name="psB", bufs=3, space="PSUM"))
        psR = ctx.enter_context(tc.tile_pool(name="psR", bufs=1, space="PSUM"))
        dram = ctx.enter_context(tc.tile_pool(name="dram", bufs=1, space="DRAM"))

        # resident constants; wqk + the first x chunk first so the PE can
        # start as early as possible
        # split the first DMAs so they spread across queues and the first
        # matmul group can start as early as possible
        wqk_sb = const.tile([128, KT, 2 * HSC * 128], bf16)
        for ct in range(2 * HSC):
            nc.sync.dma_start(wqk_sb[:, :, ct * 128:(ct + 1) * 128],
                              wqk_rs[:, :, ct * 128:(ct + 1) * 128])
        xt0_sb = stream.tile([128, KT, 512], bf16, tag="stream", name="xt0")
        nc.sync.dma_start(xt0_sb[:, 0:KT // 2, :], xt_rs[:, 0:KT // 2, 0:512])
        nc.sync.dma_start(xt0_sb[:, KT // 2:, :], xt_rs[:, KT // 2:, 0:512])
        cos_sb = const.tile([128, S], bf16)
        nc.sync.dma_start(cos_sb[:], cost[:])
        sins_sb = const.tile([128, S], bf16)
        nc.sync.dma_start(sins_sb[:], sins[:])
        wv_sb = const.tile([128, KT, HSC * 128], bf16)
        nc.sync.dma_start(wv_sb[:], wv_rs[:])
        pmA_sb = const.tile([128, 2, 512], bf16)
        nc.sync.dma_start(pmA_sb[:], pmA.rearrange("p (t q) -> p t q", t=2))
        pmB_sb = const.tile([128, 2, 512], bf16)
        nc.sync.dma_start(pmB_sb[:], pmB.rearrange("p (t q) -> p t q", t=2))
        ones_sb = const.tile([128, 1], bf16)
        nc.sync.dma_start(ones_sb[:], ones[:])

        # normalized attention output, transposed: [d, head, b*S+s] (bf16)
        attnT_sb = const.tile([128, HSC, S2], bf16)

        # the diagonal tile-pair uses a column-restricted exp, so its exp
        # buffers are partially stale; zero them once so stale bits can
        # never be NaN/Inf patterns (the pair mask multiplies them to 0)
        for i in range(6):
            exz = expp.tile([128, 2, 512], bf16, tag="ex", name=f"exz{i}")
            nc.any.memset(exz[:], 0.0)

        a2a_outs = []
        for b in range(B):
            # -------- QKV projection for batch b (heads of this core) -----
            q_sb = [qkvp.tile([128, S], bf16, tag=f"q{h}", name=f"q{h}_{b}")
                    for h in range(HSC)]
            k_sb = [qkvp.tile([128, S], bf16, tag=f"k{h}", name=f"k{h}_{b}")
                    for h in range(HSC)]
            v_sb = qkvp.tile([128, S // 128, HSC * 128], bf16, tag="v")

            for e4 in range(S // 512):
                s0 = e4 * 512
                if b == 0 and e4 == 0:
                    xt_sb = xt0_sb
                else:
                    xt_sb = stream.tile([128, KT, 512], bf16, tag="stream")
                    nc.sync.dma_start(
                        xt_sb[:], xt_rs[:, :, b * S + s0:b * S + s0 + 512])

                # q/k tiles: out^T layout [c, s], N=512
                for ct in range(2 * HSC):
                    ps = psA.tile([128, 2, 512], f32, tag="psA",
                                  name="ps_qk")[:, 0, :]
                    for kt in range(KT):
                        nc.tensor.matmul(
                            ps,
                            wqk_sb[:, kt, ct * 128:(ct + 1) * 128],
                            xt_sb[:, kt, :],
                            start=(kt == 0), stop=(kt == KT - 1),
                        )
                    dst = q_sb[ct] if ct < HSC else k_sb[ct - HSC]
                    sl = slice(s0, s0 + 512)
                    # RoPE: dst = ps*cos + swap_half(ps)*sign_sin
                    t1 = tmp.tile([128, 512], f32, tag="ropetmp")
                    nc.vector.tensor_mul(t1[:], ps, cos_sb[:, sl])
                    t2 = tmp.tile([128, 512], f32, tag="ropetmp2")
                    nc.vector.tensor_mul(t2[0:64, :], ps[64:128, :],
                                         sins_sb[0:64, sl])
                    nc.vector.tensor_mul(t2[64:128, :], ps[0:64, :],
                                         sins_sb[64:128, sl])
                    nc.vector.tensor_add(dst[:, sl], t1[:], t2[:])

                # v tiles: natural [s, c] layout
                for sv in range(4):
                    psv = psB.tile([128, 512], f32, tag="psB",
                                   name="psv")[:, 0:HSC * 128]
                    for kt in range(KT):
                        nc.tensor.matmul(
                            psv,
                            xt_sb[:, kt, sv * 128:(sv + 1) * 128],
                            wv_sb[:, kt, :],
                            start=(kt == 0), stop=(kt == KT - 1),
                        )
                    nc.scalar.copy(v_sb[:, e4 * 4 + sv, :], psv)

            # -------- causal attention for batch b --------
            # one AllToAll per head (head-outer loop), so the first
            # collective overlaps the second head's attention and only a
            # half-size collective remains exposed at the batch tail.
            # The very last piece (batch 1, head 1) is further split into
            # two quarter-size collectives by sequence half, so the final
            # o-proj can start on the first half while the second flies.
            pieces = [(0, 0, SL8), (1, 0, SL8)]
            a_ins = [dram.tile([n_cores, 128, p1 - p0], bf16,
                               name=f"a2a_in_{b}_{hh}_{p0}")
                     for hh, p0, p1 in pieces]
            a_outs = [dram.tile([n_cores, 128, p1 - p0], bf16,
                                name=f"a2a_out_{b}_{hh}_{p0}")
                      for hh, p0, p1 in pieces]

            # The whole softmax-normalize chain (rowsum copy, reciprocal,
            # GpSimd partition-broadcast, DVE multiply) runs off the PE, so
            # the PE pipeline is a flat stream of score / attn@V / rowsum
            # matmuls across all chunks and heads, two tile-pairs deep.
            deferred = [None]
            state = {}
            pend = []

            def emit_norm(bb=b, pieces=pieces, a_ins=a_ins, a_outs=a_outs):
                av, bcs, c, h = deferred[0]
                deferred[0] = None
                c0 = bb * S + c * 512
                nc.vector.tensor_mul(
                    attnT_sb[:, h, c0:c0 + 512], av, bcs[:])
                for pc, (hh, p0, p1) in enumerate(pieces):
                    if hh != h:
                        continue
                    for jj in (2 * c, 2 * c + 1):
                        s0 = bb * S + jj * SL8
                        nc.sync.dma_start(
                            a_ins[pc][jj, :, :],
                            attnT_sb[:, h, s0 + p0:s0 + p1],
                        )
                    if c == NCH - 1:
                        nc.gpsimd.collective_compute(
                            "AllToAll",
                            mybir.AluOpType.bypass,
                            replica_groups=[list(range(n_cores))],
                            ins=[a_ins[pc].opt()],
                            outs=[a_outs[pc].opt()],
                        )

            def flush_one():
                # attn@V + rowsum matmuls for the oldest pending tile pair
                # (two pairs behind the score matmuls, so the PE never waits
                # on the ScalarE exp + DVE pair-sum chain)
                ex, spt, pi, c, h, islast = pend.pop(0)
                if pi == 0:
                    state[(c, h)] = [
                        psB.tile([128, 512], f32, tag="psB", name="av"),
                        None,
                    ]
                if pi == 1:
                    state[(c, h)][1] = psR.tile([1, 512], f32, tag="rs",
                                                name="rs")
                av, rs = state[(c, h)]
                if islast:
                    # diagonal pair: columns < 256 are causally zero
                    nc.tensor.matmul(
                        av[:, 256:512], v_sb[:, 2 * pi, h * 128:(h + 1) * 128],
                        ex[:, 0, 256:512], start=False, stop=False)
                    nc.tensor.matmul(
                        av[:, 256:512],
                        v_sb[:, 2 * pi + 1, h * 128:(h + 1) * 128],
                        ex[:, 1, 256:512], start=False, stop=True)
                else:
                    nc.tensor.matmul(
                        av, v_sb[:, 2 * pi, h * 128:(h + 1) * 128],
                        ex[:, 0, :], start=(pi == 0), stop=False)
                    nc.tensor.matmul(
                        av, v_sb[:, 2 * pi + 1, h * 128:(h + 1) * 128],
                        ex[:, 1, :], start=False, stop=False)
                if spt is not None:
                    nc.tensor.matmul(
                        rs, ones_sb[:], spt[:],
                        start=(pi == 1), stop=islast)
                if islast:
                    rs_sb = tmp.tile([1, 512], f32, tag="rs_sb")
                    nc.vector.tensor_copy(rs_sb[:], rs[:])
                    rcp32 = tmp.tile([1, 512], f32, tag="rcp32")
                    nc.vector.reciprocal_approx_fast(rcp32[:], rs_sb[:])
                    bcs = tmp.tile([128, 512], f32, tag="bcs")
                    nc.gpsimd.partition_broadcast(bcs[:], rcp32[:])
                    if deferred[0] is not None:
                        emit_norm()
                    deferred[0] = (av, bcs, c, h)

            for h in range(HSC):
                qh, kh = q_sb[h], k_sb[h]
                for c in range(NCH):
                    qsl = slice(c * 512, (c + 1) * 512)
                    nkt = 4 * c + 4
                    npair = nkt // 2
                    last_pt = [None]
                    for pi in range(npair):
                        kt2 = 2 * pi
                        sc = psA.tile([128, 2, 512], f32, tag="psA", name="sc")
                        nc.tensor.matmul(
                            sc[:, 0, :],
                            kh[:, kt2 * 128:(kt2 + 1) * 128], qh[:, qsl])
                        nc.tensor.matmul(
                            sc[:, 1, :],
                            kh[:, (kt2 + 1) * 128:(kt2 + 2) * 128], qh[:, qsl])
                        if len(pend) == 3:
                            flush_one()
                        ex = expp.tile([128, 2, 512], bf16, tag="ex")
                        if pi == npair - 1:
                            # diagonal pair B: columns < 256 causally zero;
                            # restricted exp + restricted pair mask (the
                            # stale low half is never read)
                            nc.scalar.activation(ex[:, :, 256:512],
                                                 sc[:, :, 256:512],
                                                 Exp, scale=hd_scale)
                            nc.vector.tensor_mul(ex[:, :, 256:512],
                                                 ex[:, :, 256:512],
                                                 pmB_sb[:, :, 256:512])
                        elif pi == npair - 2:
                            nc.scalar.activation(ex[:], sc[:], Exp,
                                                 scale=hd_scale)
                            nc.vector.tensor_mul(ex[:], ex[:], pmA_sb[:])
                        else:
                            nc.scalar.activation(ex[:], sc[:], Exp,
                                                 scale=hd_scale)
                        # denominator tree (DVE): every odd pair sums the
                        # two exp pairs in one wide op, then folds the two
                        # tile-halves into the super-pair sum feeding the
                        # PE rowsum matmul
                        spt = None
                        if pi % 2 == 0:
                            last_pt[0] = ex
                        else:
                            lex = last_pt[0]
                            tp = ptp.tile([128, 2, 512], bf16, tag="tp")
                            if pi == npair - 1:
                                nc.vector.tensor_copy(tp[:, :, 0:256],
                                                      lex[:, :, 0:256])
                                nc.vector.tensor_add(tp[:, :, 256:512],
                                                     lex[:, :, 256:512],
                                                     ex[:, :, 256:512])
                            else:
                                nc.vector.tensor_add(tp[:], lex[:], ex[:])
                            spt = sptp.tile([128, 512], bf16, tag="spt")
                            nc.vector.tensor_add(spt[:], tp[:, 0, :],
                                                 tp[:, 1, :])
                        pend.append((ex, spt, pi, c, h, pi == npair - 1))
            while pend:
                flush_one()
            # last chunk/head: emit its normalize (and its collective) now
            emit_norm()
            a2a_outs.append((pieces, a_outs))

        # -------- output projection for this core's row slices --------
        # core's out rows: [0:SL8] = batch-0 slice, [SL8:2*SL8] = batch-1
        # batch-outer so the batch-1 gather (which waits on the second
        # AllToAll) never blocks batch-0's W_o loads in the Sync DMA queue.
        # Batch 1 walks ec in reverse: ec3/ec2 W_o tiles are still resident
        # from batch 0 (wop bufs=3), so its first matmuls only wait on the
        # gather; ec1/ec0 reload in the shadow of ec3/ec2 compute.
        STB = max(1, SL8 // 128)
        PS = min(128, SL8)
        wo_tiles = {}

        def load_wo(ec, name):
            wo_sb = wop.tile([128, H, 512], bf16, tag="wo", name=name)
            nc.sync.dma_start(wo_sb[:], wo_rs[:, :, ec * 512:(ec + 1) * 512])
            wo_tiles[ec] = wo_sb

        # accumulate first-head (even) rows first so the second head's
        # gather gets extra slack
        ht_order = [r * HSC for r in range(n_cores)] + \
                   [r * HSC + 1 for r in range(n_cores)]

        def po_group(b, ec, st, atn_sb):
            po = psB.tile([128, 512], f32, tag="psB", name="po")[:PS]
            wo_sb = wo_tiles[ec]
            for hi, ht in enumerate(ht_order):
                nc.tensor.matmul(
                    po,
                    atn_sb[:, ht // HSC, ht % HSC, st * 128:st * 128 + PS],
                    wo_sb[:, ht, :],
                    start=(hi == 0), stop=(hi == H - 1),
                )
            ot = tmp.tile([128, 512], f32, tag="ot")
            nc.scalar.copy(ot[:PS, :], po)
            r0 = b * SL8 + st * 128
            nc.sync.dma_start(out[r0:r0 + PS, ec * 512:(ec + 1) * 512],
                              ot[:PS, :])

        def gather(b, pc, atn_sb):
            hh, p0, p1 = a2a_outs[b][0][pc]
            nc.sync.dma_start(atn_sb[:, :, hh, p0:p1],
                              a2a_outs[b][1][pc].rearrange("r p s -> p r s"))

        # batch 0: plain ec-major order
        atn0 = stream.tile([128, n_cores, HSC, SL8], bf16, tag="stream",
                           name="atn_0")
        gather(0, 0, atn0)
        gather(0, 1, atn0)
        for ec in range(ECH):
            load_wo(ec, f"wo_b0_{ec}")
            for st in range(STB):
                po_group(0, ec, st, atn0)
        # batch 1: ec3/ec2 W_o still resident from batch 0; st=0 rows only
        # need the first half-gather of head 1, so they run while the last
        # quarter-size collective is still in flight
        atn1 = stream.tile([128, n_cores, HSC, SL8], bf16, tag="stream",
                           name="atn_1")
        gather(1, 0, atn1)           # head 0
        load_wo(1, "wo_b1_1")
        gather(1, 1, atn1)           # head 1
        for ec in (3, 2):
            for st in range(STB):
                po_group(1, ec, st, atn1)
        load_wo(0, "wo_b1_0")
        for ec in (1, 0):
            for st in range(STB):
                po_group(1, ec, st, atn1)

    nc.finalize()
    return nc


# ---------------------------------------------------------------- host code
def make_tables(S):
    half = HD // 2
    inv_freq = (1.0 / (10000.0 ** (np.arange(half, dtype=np.float32) / half)))
    pos = np.arange(S, dtype=np.float32)
    freqs = pos[:, None] * inv_freq[None, :]          # [S, half]
    cos = np.cos(freqs).astype(np.float32)            # [S, half]
    sin = np.sin(freqs).astype(np.float32)
    cosT = np.concatenate([cos, cos], axis=1).T       # [HD, S]
    # sign-folded sin: rows 0..63 get -sin, rows 64..127 get +sin
    sinsT = np.concatenate([-sin, sin], axis=1).T     # [HD, S]
    return np.ascontiguousarray(cosT), np.ascontiguousarray(sinsT)


def make_mask():
    j = np.arange(896)[None, :]
    k = np.arange(128)[:, None]
    return ((j - 384) >= k).astype(np.float32)        # [128, 896]


def make_pair_masks():
    m = make_mask()
    pmA = np.concatenate([m[:, 384:896], m[:, 256:768]], axis=1)  # [128,1024]
    pmB = np.concatenate([m[:, 128:640], m[:, 0:512]], axis=1)
    return pmA, pmB


def prepare_in_maps(x, W_qkv, W_o, S, D):
    import ml_dtypes
    bf16 = ml_dtypes.bfloat16

    S2 = B * S
    xT = np.ascontiguousarray(
        x.reshape(S2, D).T.astype(np.float32)).astype(bf16)
    cosT, sinsT = make_tables(S)
    pmA, pmB = make_pair_masks()
    ones = np.ones((128, 1), bf16)
    wo_bf16 = W_o.astype(bf16)

    qw = W_qkv[:, 0 * H * HD:1 * H * HD]
    kw = W_qkv[:, 1 * H * HD:2 * H * HD]
    vw = W_qkv[:, 2 * H * HD:3 * H * HD]

    in_maps = []
    for c in range(N_CORES):
        h0 = c * HEADS_PER_CORE
        cols = slice(h0 * HD, (h0 + HEADS_PER_CORE) * HD)
        wqk_c = np.ascontiguousarray(
            np.concatenate([qw[:, cols], kw[:, cols]], axis=1)).astype(bf16)
        wv_c = np.ascontiguousarray(vw[:, cols]).astype(bf16)
        in_maps.append({
            "xt": xT, "wqk": wqk_c, "wv": wv_c, "wo": wo_bf16,
            "cost": cosT.astype(bf16), "sins": sinsT.astype(bf16),
            "pmA": pmA.astype(bf16), "pmB": pmB.astype(bf16), "ones": ones,
        })
    return in_maps


_NC_CACHE = {}


def run(x, W_qkv, W_o, S, D, trace=False, trace_kwargs=None):
    from concourse.bass_utils import run_bass_kernel_spmd

    key = (S, D)
    if key not in _NC_CACHE:
        _NC_CACHE[key] = build_nc(S=S, D=D)
    nc = _NC_CACHE[key]
    in_maps = prepare_in_maps(x, W_qkv, W_o, S, D)
    res = run_bass_kernel_spmd(
        nc, in_maps, core_ids=list(range(N_CORES)),
        trace=trace, **(trace_kwargs or {}),
    )
    SL8 = S // N_CORES
    full = np.empty((B, S, D), np.float32)
    for c in range(N_CORES):
        o = res.results[c]["out"]
        full[0, c * SL8:(c + 1) * SL8] = o[:SL8]
        full[1, c * SL8:(c + 1) * SL8] = o[SL8:]
    return full, res


def kernel(x, W_qkv, W_o):
    x = np.asarray(x)
    W_qkv = np.asarray(W_qkv)
    W_o = np.asarray(W_o)
    S, D = x.shape[1], x.shape[2]
    out, _ = run(x, W_qkv, W_o, S, D, trace=False)
    return out.astype(np.float32)
